# revision 44
# baseline (speedup 1.0000x reference)
"""MoE layer (8 experts, top-2) on 8 TRN2 NeuronCores, expert-parallel.

Strategy (sparse dispatch + mixed-precision mm2):
  - Host computes the router (fp32 logits, top-2, softmax renormalize) and
    dispatches: core m owns expert m's weights.
  - Per expert, tokens sort by combine weight. The G=640 highest-weight
    token-expert pairs run fully in bf16. The remainder (low combine
    weight) runs mm1 in bf16 but mm2 in fp8 e4m3 with DoubleRow perf mode
    (2x PE throughput); the fp8 quantization error is suppressed by those
    tokens' small combine weights (overall rel-err ~1.84e-2 < 2e-2,
    verified bit-close between numpy sim and hardware).
  - SPMD static shapes per core: G bf16 tokens + 384-token fp8 slot A
    (own expert) + 64-token fp8 slot B (another expert's overflow, that
    expert's weights are DMA'd to this core). Underfilled slots pad with
    zero-weight tokens.
  - Schedule details tuned against the TimelineSim cost model: PE warmup
    matmuls during the initial DMA wait (p-state ramp), 1MB weight tiles
    (DMA deps are whole-tile), late weights stream into slots freed by
    dead bf16 tiles in need order, cB's tiny mm1 interleaves into
    cA-mm2's psum groups, merged per-tile fp8 output DMAs.
  - Combine weights apply on device (DVE); b2 is added on host (it only
    multiplies combine weights, which the host has).
"""

from contextlib import ExitStack

import ml_dtypes
import numpy as np

P = 128
B, S, H, F, E = 2, 2048, 1024, 4096, 8
T = B * S            # 4096 tokens
KH = H // P          # 8   k-subtiles over H
KF = F // P          # 32  k-subtiles over F

G_DEF = 640          # bf16 tokens per core (multiple of 128)
FA_DEF = 384         # fp8 slot A capacity (own expert, multiple of 64)
FB_DEF = 32          # fp8 slot B capacity (spill expert)

bf16 = ml_dtypes.bfloat16
f8 = ml_dtypes.float8_e4m3fn
WSCALE = 64.0        # fp8 w2 pre-scale (folded into combine weights)

_CACHE = {}
WARMUP = 20


def _build_nc(G, FA, FB):
    import concourse.mybir as mybir
    import concourse.tile as tile
    from concourse import bacc

    dt = mybir.dt
    AF = mybir.ActivationFunctionType
    DR = mybir.MatmulPerfMode.DoubleRow

    GT = G // P                    # bf16 token tiles (6)
    NT = G + FA + FB               # tokens per core (1088)
    NAT = FA // 64                 # fp8 A mm2 tiles (4)
    NBT = (FB + 63) // 64          # fp8 B mm2 tiles (1)

    nc = bacc.Bacc(
        "TRN2", target_bir_lowering=False, debug=False, num_devices=E)

    xb = nc.declare_dram_parameter("xb", [H, NT], dt.bfloat16, isOutput=False)
    w1b = nc.declare_dram_parameter("w1b", [H, F], dt.bfloat16, isOutput=False)
    w2b = nc.declare_dram_parameter("w2b", [F, H], dt.bfloat16, isOutput=False)
    w1s = nc.declare_dram_parameter("w1s", [H, F], dt.bfloat16, isOutput=False)
    w2a = nc.declare_dram_parameter("w2a", [F, H], dt.float8e4, isOutput=False)
    w2s = nc.declare_dram_parameter("w2s", [F, H], dt.float8e4, isOutput=False)
    b1o = nc.declare_dram_parameter("b1o", [P, KF], dt.float32, isOutput=False)
    b1s = nc.declare_dram_parameter("b1s", [P, KF], dt.float32, isOutput=False)
    wmb = nc.declare_dram_parameter("wmb", [P, GT], dt.float32, isOutput=False)
    wma = nc.declare_dram_parameter("wma", [64, NAT], dt.float32, isOutput=False)
    wms = nc.declare_dram_parameter("wms", [64, NBT], dt.float32, isOutput=False)
    yc = nc.declare_dram_parameter("yc", [NT, H], dt.float32, isOutput=True)

    xb_r = xb.rearrange("(k p) t -> p k t", p=P)
    w1b_r = w1b.rearrange("(k p) f -> p k f", p=P)
    w2b_r = w2b.rearrange("(k p) h -> p k h", p=P)
    w1s_r = w1s.rearrange("(k p) f -> p k f", p=P)
    w2a_r = w2a.rearrange("(k p) h -> p k h", p=P)
    w2s_r = w2s.rearrange("(k p) h -> p k h", p=P)

    with ExitStack() as ctx:
        tc = ctx.enter_context(tile.TileContext(nc))
        const = ctx.enter_context(tc.tile_pool(name="const", bufs=1))
        w1pool = ctx.enter_context(tc.tile_pool(name="w1", bufs=8))
        w2pool = ctx.enter_context(tc.tile_pool(name="w2", bufs=2))
        xbp = ctx.enter_context(tc.tile_pool(name="xb", bufs=2))
        h1bp = ctx.enter_context(tc.tile_pool(name="h1b", bufs=1))
        h1fp = ctx.enter_context(tc.tile_pool(name="h1f", bufs=1))
        h1fbp = ctx.enter_context(tc.tile_pool(name="h1fb", bufs=1))
        opool = ctx.enter_context(tc.tile_pool(name="ob", bufs=4))
        obfp = ctx.enter_context(tc.tile_pool(name="obf", bufs=2))
        p1 = ctx.enter_context(tc.tile_pool(name="p1", bufs=5, space="PSUM"))
        p2 = ctx.enter_context(tc.tile_pool(name="p2", bufs=3, space="PSUM"))

        # ---- PE warmup ----
        # The cost model ramps PE 0.65->1.2->2.4 GHz over the first 3us of
        # continuous execution. Fill the initial DMA wait (~6us) with dummy
        # matmuls on memset data so real matmuls start at full clock.
        wrm = const.tile([P, 256], dt.bfloat16)
        nc.vector.memset(wrm[:], 0.0)
        for i in range(WARMUP):
            pw = p1.tile([P, 512], dt.float32, name="ps1")[:, :256]
            nc.tensor.matmul(pw[:], wrm[:, :P], wrm[:],
                             start=True, stop=True)

        C0 = G - 256                   # bf16 chunk sizes (multiples of 128)
        C1 = 256

        # ---- x chunk loads ----
        def load_x(t0, csz):
            xt = xbp.tile([P, KH, 512], dt.bfloat16, name="xbt")[:, :, :csz]
            nc.sync.dma_start(xt[:], xb_r[:, :, t0:t0 + csz])
            return xt

        # Startup order: x chunk-0's k=0 plane, then w1b's first f-tile
        # (small, so the first real matmul is runnable after ~0.4MB), then
        # b1o for the first activation, then the rest.
        xc0 = xbp.tile([P, KH, 512], dt.bfloat16, name="xbt")[:, :, :C0]
        nc.sync.dma_start(xc0[:], xb_r[:, :, 0:C0])
        w1h0 = const.tile([P, KH, 128], dt.bfloat16)
        nc.sync.dma_start(w1h0[:], w1b_r[:, :, 0:128])
        b1o_s = const.tile([P, KF], dt.float32)
        nc.sync.dma_start(b1o_s[:], b1o[:])
        w1h1 = const.tile([P, KH, 384], dt.bfloat16)
        nc.sync.dma_start(w1h1[:], w1b_r[:, :, 128:512])

        # w1 weights: 1MB tiles so matmuls march at 1MB granularity.
        # w1b covers f 512:4096 in q1..q7 (f 0:512 lives in w1h0/w1h1).
        # Low-urgency consts ride after the first two q tiles.
        w1b_q = []
        b1s_s = wmb_s = wma_s = wms_s = None
        for q in range(1, 8):
            t = w1pool.tile([P, KH, 512], dt.bfloat16, name="w1")
            w1b_q.append(t)
            nc.sync.dma_start(t[:], w1b_r[:, :, q * 512:(q + 1) * 512])
            if q == 2:
                b1s_s = const.tile([P, KF], dt.float32)
                nc.sync.dma_start(b1s_s[:], b1s[:])
                wmb_s = const.tile([P, GT], dt.float32)
                nc.sync.dma_start(wmb_s[:], wmb[:])
                wma_s = const.tile([64, NAT], dt.float32)
                nc.sync.dma_start(wma_s[:], wma[:])
                wms_s = const.tile([64, NBT], dt.float32)
                nc.sync.dma_start(wms_s[:], wms[:])
        w2b_t = []
        for half in range(2):
            t = w2pool.tile([P, KF, H // 2], dt.bfloat16, name="w2")
            w2b_t.append(t)
            for hc in range(2):
                nc.sync.dma_start(
                    t[:, :, hc * 256:(hc + 1) * 256],
                    w2b_r[:, :, half * (H // 2) + hc * 256:half * (H // 2) + (hc + 1) * 256])

        # Remaining x chunks: issued ahead of the slot-blocked weight DMAs
        # so they don't queue behind them. Pool rotation: xc1 -> slot 1,
        # xca -> slot 0 (waits c0-mm1), xcb -> slot 1 (waits c1-mm1).
        xc1 = load_x(C0, C1)
        xca = load_x(G, FA)
        xcb = load_x(G + FA, FB)

        # ---- late weights; slot reuse + FIFO order hides their DMA ----
        # w2a reuses w2b half1's slot (dead ~27us before bf16 end), w1s
        # chunks reuse w1b's slots (dead progressively during cA-mm1),
        # w2s reuses w2b half2's slot; issue order matches need order.
        w2a_t = w2pool.tile([P, KF, H], dt.float8e4, name="w2")
        for hc in range(2):
            nc.sync.dma_start(w2a_t[:, :, hc * 512:(hc + 1) * 512],
                              w2a_r[:, :, hc * 512:(hc + 1) * 512])
        w2s_t = w2pool.tile([P, KF, H], dt.float8e4, name="w2")
        for hc in range(2):
            nc.sync.dma_start(w2s_t[:, :, hc * 512:(hc + 1) * 512],
                              w2s_r[:, :, hc * 512:(hc + 1) * 512])
        w1s_q = []
        for q in range(8):
            t = w1pool.tile([P, KH, 512], dt.bfloat16, name="w1")
            w1s_q.append(t)
            nc.sync.dma_start(t[:], w1s_r[:, :, q * 512:(q + 1) * 512])

        # ---- mm1 (bf16) ----
        def w1b_src(ft):
            if ft == 0:
                return w1h0, 0
            if ft < 4:
                return w1h1, (ft - 1) * P
            return w1b_q[ft // 4 - 1], (ft % 4) * P

        def w1s_src(ft):
            return w1s_q[ft // 4], (ft % 4) * P

        def mm1(xt, csz, src_fn, b1t, h1):
            for ft in range(KF):
                src, c0 = src_fn(ft)
                ps = p1.tile([P, 512], dt.float32, name="ps1")[:, :csz]
                for k in range(KH):
                    nc.tensor.matmul(
                        ps[:], src[:, k, c0:c0 + P], xt[:, k],
                        start=(k == 0), stop=(k == KH - 1))
                nc.scalar.activation(h1[:, ft], ps[:], AF.Gelu,
                                     bias=b1t[:, ft:ft + 1])

        # ---- mm2 bf16 ----
        def mm2_bf16(h1, t0, csz):
            # hh-major so w2b half-tiles die as early as possible (their
            # slots host the fp8 w2 loads).
            for hh in range(2):
                src = w2b_t[hh]
                for ct in range(csz // P):
                    gt = t0 // P + ct
                    ps2 = p2.tile([P, 512], dt.float32, name="ps2")
                    for kf in range(KF):
                        nc.tensor.matmul(
                            ps2[:], h1[:, kf, ct * P:(ct + 1) * P], src[:, kf, :],
                            start=(kf == 0), stop=(kf == KF - 1))
                    ob = opool.tile([P, 512], dt.float32, name="ob")
                    nc.vector.tensor_scalar_mul(ob[:], ps2[:], wmb_s[:, gt:gt + 1])
                    nc.sync.dma_start(
                        yc[gt * P:(gt + 1) * P, hh * 512:(hh + 1) * 512], ob[:])

        # ---- mm2 fp8 (DoubleRow) ----
        def mm2_fp8(h1, w2t, wmt, t0, csz, fill=None, tail_split=False):
            # fill: optional generator yielding thunks of extra PE work to
            # interleave between psum groups (keeps PE fed when the groups
            # alone would be drained faster than ACT/DVE latency allows).
            # One merged output DMA per 64-token tile (4 psum groups) keeps
            # the SP sequencer's serial DMA-issue cost off the critical path;
            # tail_split flushes the last tile's first 3 groups early so the
            # final DMA is small.
            ntiles = (csz + 63) // 64
            for tt in range(ntiles):
                rows = min(64, csz - tt * 64)
                ob = obfp.tile([64, H], dt.float32, name="obf")[:rows, :]
                last_tile = tt == ntiles - 1
                for hh in range(4):
                    ps2 = p2.tile([P, 512], dt.float32, name="ps2")[:rows, :256]
                    for kp in range(KF // 2):
                        nc.tensor.matmul(
                            ps2[:], h1[:, 2 * kp:2 * kp + 2, tt * 64:tt * 64 + rows],
                            w2t[:, 2 * kp:2 * kp + 2, hh * 256:(hh + 1) * 256],
                            start=(kp == 0), stop=(kp == KF // 2 - 1),
                            perf_mode=DR)
                    nc.vector.tensor_scalar_mul(
                        ob[:, hh * 256:(hh + 1) * 256], ps2[:], wmt[:rows, tt:tt + 1])
                    if tail_split and last_tile and hh == 2:
                        nc.sync.dma_start(
                            yc[t0 + tt * 64:t0 + tt * 64 + rows, 0:768],
                            ob[:, 0:768])
                    if fill is not None:
                        for thunk in fill(tt * 4 + hh):
                            thunk()
                if tail_split and last_tile:
                    nc.sync.dma_start(
                        yc[t0 + tt * 64:t0 + tt * 64 + rows, 768:H],
                        ob[:, 768:H])
                else:
                    nc.sync.dma_start(
                        yc[t0 + tt * 64:t0 + tt * 64 + rows, :], ob[:])

        # ---- schedule ----
        h1 = h1bp.tile([P, KF, C0], dt.bfloat16, name="h1b")
        mm1(xc0, C0, w1b_src, b1o_s, h1)
        mm2_bf16(h1, 0, C0)

        h1 = h1bp.tile([P, KF, C0], dt.bfloat16, name="h1b")[:, :, :C1]
        mm1(xc1, C1, w1b_src, b1o_s, h1)
        mm2_bf16(h1, C0, C1)

        h1a = h1fp.tile([P, KF, FA], dt.float8e4, name="h1f")
        mm1(xca, FA, w1b_src, b1o_s, h1a)

        # cB's tiny mm1 is ACT-latency-paced on its own (22ns PE bubbles
        # that also reset the p-state ramp); interleave its f-tiles
        # between cA-mm2 psum groups so PE stays saturated.
        h1b2 = h1fbp.tile([P, KF, FB], dt.float8e4, name="h1fb")

        def emit_cb_ft(ft):
            src, c0 = w1s_src(ft)
            ps = p1.tile([P, 512], dt.float32, name="ps1")[:, :FB]
            for k in range(KH):
                nc.tensor.matmul(
                    ps[:], src[:, k, c0:c0 + P], xcb[:, k],
                    start=(k == 0), stop=(k == KH - 1))
            nc.scalar.activation(h1b2[:, ft], ps[:], AF.Gelu,
                                 bias=b1s_s[:, ft:ft + 1])

        def fill_cb(group_idx):
            ft0 = group_idx * 2
            return [(lambda ft=ft: emit_cb_ft(ft))
                    for ft in range(ft0, min(ft0 + 2, KF))]

        mm2_fp8(h1a, w2a_t, wma_s, G, FA, fill=fill_cb)
        mm2_fp8(h1b2, w2s_t, wms_s, G + FA, FB, tail_split=True)
    return nc


def _get_nc(G, FA, FB):
    key = (G, FA, FB)
    if key not in _CACHE:
        nc = _build_nc(G, FA, FB)
        nc.finalize()
        _CACHE[key] = nc
    return _CACHE[key]


def dispatch(hidden_states, router_w, router_b):
    """Host router: top-2 ids + renormalized combine weights per token."""
    x = np.asarray(hidden_states, dtype=np.float32).reshape(T, H)
    logits = x @ np.asarray(router_w, dtype=np.float32)
    logits = logits + np.asarray(router_b, dtype=np.float32)
    part = np.argpartition(logits, E - 2, axis=1)[:, E - 2:]      # [T,2] unordered
    pv = np.take_along_axis(logits, part, axis=1)
    swap = pv[:, 0] > pv[:, 1]
    i1 = np.where(swap, part[:, 0], part[:, 1])
    i2 = np.where(swap, part[:, 1], part[:, 0])
    l1 = logits[np.arange(T), i1]
    l2 = logits[np.arange(T), i2]
    e2 = np.exp((l2 - l1).astype(np.float64))
    wt1 = (1.0 / (1.0 + e2)).astype(np.float32)
    wt2 = (e2 / (1.0 + e2)).astype(np.float32)
    return x, i1, i2, wt1, wt2


def plan(i1, i2, wt1, wt2, G, FA, FB):
    """Token->(core, group) assignment."""
    bf_tok, bf_wt, a_tok, a_wt, spill = [], [], [], [], []
    for m in range(E):
        tk = np.concatenate([np.where(i1 == m)[0], np.where(i2 == m)[0]])
        wt = np.concatenate([wt1[i1 == m], wt2[i2 == m]])
        o = np.argsort(-wt)
        tk, wt = tk[o], wt[o]
        bf_tok.append(tk[:G])
        bf_wt.append(wt[:G])
        a_tok.append(tk[G:G + FA])
        a_wt.append(wt[G:G + FA])
        rest_t, rest_w = tk[G + FA:], wt[G + FA:]
        for s0 in range(0, len(rest_t), FB):
            spill.append((m, rest_t[s0:s0 + FB], rest_w[s0:s0 + FB]))
    assert len(spill) <= E, f"spill slots {len(spill)} > {E}"
    while len(spill) < E:
        spill.append((0, np.zeros(0, np.int64), np.zeros(0, np.float32)))
    return bf_tok, bf_wt, a_tok, a_wt, spill


def make_in_maps(hidden_states, router_w, router_b, w1, b1, w2, b2,
                 G=G_DEF, FA=FA_DEF, FB=FB_DEF):
    x, i1, i2, wt1, wt2 = dispatch(hidden_states, router_w, router_b)
    bf_tok, bf_wt, a_tok, a_wt, spill = plan(i1, i2, wt1, wt2, G, FA, FB)
    w1 = np.asarray(w1, dtype=np.float32)
    w2 = np.asarray(w2, dtype=np.float32)
    b1 = np.asarray(b1, dtype=np.float32)
    b2 = np.asarray(b2, dtype=np.float32)
    xt = np.ascontiguousarray(x.T)                      # [H, T]
    GT = G // P

    def wcol(wts, cap, rows, scale):
        ncol = cap // rows
        out = np.zeros((rows, ncol), dtype=np.float32)
        wv = np.zeros(cap, dtype=np.float32)
        wv[:len(wts)] = wts * scale
        for c in range(ncol):
            out[:, c] = wv[c * rows:(c + 1) * rows]
        return out

    w1b16 = [np.ascontiguousarray(w1[m].astype(bf16)) for m in range(E)]
    w2f8 = [np.ascontiguousarray((w2[m] * WSCALE).astype(f8)) for m in range(E)]
    b1r = [np.ascontiguousarray(b1[m].reshape(KF, P).T) for m in range(E)]

    in_maps = []
    for m in range(E):
        sm, st, sw = spill[m]
        xbm = np.zeros((H, G + FA + FB), dtype=bf16)
        xbm[:, :len(bf_tok[m])] = xt[:, bf_tok[m]].astype(bf16)
        xbm[:, G:G + len(a_tok[m])] = xt[:, a_tok[m]].astype(bf16)
        xbm[:, G + FA:G + FA + len(st)] = xt[:, st].astype(bf16)
        in_maps.append({
            "xb": xbm,
            "w1b": w1b16[m],
            "w2b": np.ascontiguousarray(w2[m].astype(bf16)),
            "w1s": w1b16[sm],
            "w2a": w2f8[m], "w2s": w2f8[sm],
            "b1o": b1r[m], "b1s": b1r[sm],
            "wmb": wcol(bf_wt[m], GT * P, P, 1.0),
            "wma": wcol(a_wt[m], FA, 64, 1.0 / WSCALE),
            "wms": wcol(sw, max(64, FB), 64, 1.0 / WSCALE),
        })
    meta = (bf_tok, a_tok, spill, i1, i2, wt1, wt2)
    return in_maps, meta


def run_device(in_maps, G=G_DEF, FA=FA_DEF, FB=FB_DEF):
    from concourse.bass_utils import run_bass_kernel_spmd

    nc = _get_nc(G, FA, FB)
    res = run_bass_kernel_spmd(nc, in_maps, core_ids=list(range(E)))
    return res.results


def kernel(hidden_states, router_w, router_b, w1, b1, w2, b2):
    G, FA, FB = G_DEF, FA_DEF, FB_DEF
    in_maps, meta = make_in_maps(
        hidden_states, router_w, router_b, w1, b1, w2, b2, G, FA, FB)
    bf_tok, a_tok, spill, i1, i2, wt1, wt2 = meta
    b2 = np.asarray(b2, dtype=np.float32)
    # One retry guards against rare transient NRT/axon failures.
    last_err = None
    for attempt in range(3):
        try:
            results = run_device(in_maps, G, FA, FB)
        except Exception as e:
            last_err = e
            import time as _time
            _time.sleep(10)
            continue
        acc = np.zeros((T, H), dtype=np.float32)
        for m in range(E):
            ycm = np.asarray(results[m]["yc"], dtype=np.float32)
            acc[bf_tok[m]] += ycm[:len(bf_tok[m])]
            if len(a_tok[m]):
                acc[a_tok[m]] += ycm[G:G + len(a_tok[m])]
            sm, st, sw = spill[m]
            if len(st):
                acc[st] += ycm[G + FA:G + FA + len(st)]
        # b2 contribution (combine-weighted), host-side
        acc += wt1[:, None] * b2[i1] + wt2[:, None] * b2[i2]
        if np.isfinite(acc).all() and np.abs(acc).max() < 1e4:
            return acc.reshape(B, S, H)
    if last_err is not None:
        raise last_err
    return acc.reshape(B, S, H)
